# revision 1
# baseline (speedup 1.0000x reference)
"""DeepseekV3 decoder layer on 8 Trainium2 NeuronCores (Bass/Tile).

Sharding: tensor-parallel heads for MLA (2 heads/core), expert-parallel
routed experts (1/core), token shards (256 tok/core) for everything else.
Collectives: AllGather(latents) -> AllToAll(attn out) -> AllGather(h bf16)
+ AllGather(router weights) -> ReduceScatter(expert partials, bf16).

Activations kept feature-major [feat_p, tok_f]; weights pre-transposed on
host to [K, M] so each matmul is lhsT[K,M].T @ rhs[K,N], K = contraction.
Matmuls run float32r except the router chain (fp32) and experts (bf16).
"""
import sys

if "/opt/trn_rl_repo" not in sys.path:
    sys.path.insert(0, "/opt/trn_rl_repo")

import numpy as np
import ml_dtypes

import concourse.bass as bass
import concourse.bacc as bacc
import concourse.tile as tile
from concourse import mybir
from concourse import bass_utils

FP = mybir.dt.float32
BF = mybir.dt.bfloat16
FR = mybir.dt.float32r
AF = mybir.ActivationFunctionType
ALU = mybir.AluOpType

NCORE = 8
B, S, H = 2, 1024, 2048
T = B * S
NH, DN, DR, DV = 16, 128, 64, 128
DQK = DN + DR
KVL, QL = 512, 1536
NE, NG, INTER = 8, 4, 768
TSH = T // NCORE          # 256
HPC = NH // NCORE         # 2
SCALING = float(DQK) ** -0.5
RSF = 2.5
EPS = 1e-6

AG1_ROWS = QL + KVL + DR + 2   # 2114


def fr(ap):
    return ap.bitcast(FR)


def build_program():
    nc = bacc.Bacc("TRN2", target_bir_lowering=False, debug=False,
                   num_devices=NCORE)

    def din(name, shape, dtype=FP):
        return nc.dram_tensor(name, shape, dtype, kind="ExternalInput").ap()

    hidT = din("hidT", [H, TSH])
    qa_wT = din("qa_wT", [H, QL])
    kva_wT = din("kva_wT", [H, KVL + DR])
    qb_wT = din("qb_wT", [QL, HPC * DQK])      # cols: h0n h1n h0A h0B h1A h1B
    kvb_wT = din("kvb_wT", [KVL, HPC * 256])   # cols: k0 k1 v0 v1
    o_wT = din("o_wT", [NH * DV, H])
    r_wT = din("r_wT", [H, NE])
    r_bias = din("r_bias", [NE, 1])
    onehot = din("onehot", [NE, 1])
    g_wT = din("g_wT", [H, INTER], BF)
    u_wT = din("u_wT", [H, INTER], BF)
    d_wT = din("d_wT", [INTER, H], BF)
    sg_wT = din("sg_wT", [H, INTER])
    su_wT = din("su_wT", [H, INTER])
    sd_wT = din("sd_wT", [INTER, H])
    cc_q = din("cc_q", [128, T])
    ss_q = din("ss_q", [128, T])
    cc_k = din("cc_k", [DR, TSH])
    ss_k = din("ss_k", [DR, TSH])
    maskT_d = din("maskT", [512, 512])
    Gm_d = din("Gm", [NE, NG])
    Dg_d = din("Dg", [NG, NG * NG])
    Rg_d = din("Rg", [NG * NG, NG])
    Em_d = din("Em", [NG, NE])
    De_d = din("De", [NE, NE * NE])
    Re_d = din("Re", [NE * NE, NE])

    out = nc.dram_tensor("out", [H, TSH], FP, kind="ExternalOutput").ap()

    RG = [list(range(NCORE))]

    def dma(out_ap, in_ap):
        nc.sync.dma_start(out_ap, in_ap)

    def kp(ap, p=128):
        return ap.rearrange("(k p) t -> p k t", p=p)

    tcx = tile.TileContext(nc)
    tc = tcx.__enter__()
    dram_cm = tc.tile_pool(name="dram", bufs=1, space="DRAM")
    dram = dram_cm.__enter__()
    pp_cm = tc.tile_pool(name="persist", bufs=1)
    pp = pp_cm.__enter__()

    ag1_in = dram.tile([AG1_ROWS, TSH], FP)
    ag1_out = dram.tile([NCORE * AG1_ROWS, TSH], FP, addr_space="Shared")
    a2a_in = dram.tile([NCORE * 256, TSH], FP)
    a2a_out = dram.tile([NCORE * 256, TSH], FP)
    ag2_in = dram.tile([H, TSH], BF)
    ag2_out = dram.tile([NCORE * H, TSH], BF, addr_space="Shared")
    agw_in = dram.tile([NE, TSH], FP)
    agw_out = dram.tile([NCORE * NE, TSH], FP, addr_space="Shared")
    rs_in = dram.tile([NCORE * H, TSH], BF)
    rs_out = dram.tile([H, TSH], BF)

    ones = pp.tile([128, 1], FP)
    nc.vector.memset(ones[:], 1.0)
    epsb = pp.tile([128, 1], FP)
    nc.vector.memset(epsb[:], EPS)

    ag1v = ag1_out.rearrange("(j r) t -> j r t", r=AG1_ROWS)

    # ==================== phase A: local latents ====================
    with tc.tile_pool(name="pA", bufs=1) as pa, \
         tc.tile_pool(name="pAw", bufs=3) as paw, \
         tc.tile_pool(name="pAt", bufs=2) as pat, \
         tc.tile_pool(name="psA", bufs=2, space="PSUM") as psa:

        x0 = pa.tile([128, 16, TSH], FR)
        dma(x0[:], kp(hidT).bitcast(FR))

        ss_ps = psa.tile([1, TSH], FP, tag="st")
        for k in range(16):
            sq = pat.tile([128, TSH], FR, tag="sq")
            nc.scalar.square(sq[:], x0[:, k, :])
            nc.tensor.matmul(ss_ps[:], fr(ones[:]), fr(sq[:]),
                             start=(k == 0), stop=(k == 15))
        rstd = pa.tile([1, TSH], FP)
        nc.scalar.activation(rstd[:], ss_ps[:], AF.Sqrt,
                             bias=epsb[0:1, :], scale=1.0 / H)
        nc.vector.reciprocal(rstd[:], rstd[:])
        bloc = pa.tile([128, TSH], FP)
        nc.gpsimd.partition_broadcast(bloc[:], rstd[:1, :])

        qa_s = pa.tile([128, 12, TSH], FP)
        for m in range(12):
            wa = paw.tile([128, 16, 128], FR, tag="wa")
            dma(wa[:], kp(qa_wT[:, 128 * m:128 * (m + 1)]).bitcast(FR))
            ps = psa.tile([128, TSH], FP, tag="mm")
            for k in range(16):
                nc.tensor.matmul(ps[:], fr(wa[:, k, :]), fr(x0[:, k, :]),
                                 start=(k == 0), stop=(k == 15))
            nc.vector.tensor_mul(qa_s[:, m, :], ps[:], bloc[:])

        ss2 = psa.tile([1, TSH], FP, tag="st")
        for m in range(12):
            sq = pat.tile([128, TSH], FR, tag="sq")
            nc.scalar.square(sq[:], qa_s[:, m, :])
            nc.tensor.matmul(ss2[:], fr(ones[:]), fr(sq[:]),
                             start=(m == 0), stop=(m == 11))
        r2 = pa.tile([1, TSH], FP)
        nc.scalar.activation(r2[:], ss2[:], AF.Sqrt,
                             bias=epsb[0:1, :], scale=1.0 / QL)
        nc.vector.reciprocal(r2[:], r2[:])

        ckv_s = pa.tile([128, 4, TSH], FP)
        kr_raw = pa.tile([64, TSH], FP)
        for m in range(5):
            mc = 128 if m < 4 else 64
            wv = paw.tile([128, 16, 128], FR, tag="wa")
            dma(wv[:, :, :mc], kp(kva_wT[:, 128 * m:128 * m + mc]).bitcast(FR))
            ps = psa.tile([128, TSH], FP, tag="mm")
            for k in range(16):
                nc.tensor.matmul(ps[:mc, :], fr(wv[:, k, :mc]),
                                 fr(x0[:, k, :]),
                                 start=(k == 0), stop=(k == 15))
            if m < 4:
                nc.vector.tensor_mul(ckv_s[:, m, :], ps[:], bloc[:])
            else:
                nc.vector.tensor_mul(kr_raw[:], ps[:64, :], bloc[:64, :])

        ss3 = psa.tile([1, TSH], FP, tag="st")
        for m in range(4):
            sq = pat.tile([128, TSH], FR, tag="sq")
            nc.scalar.square(sq[:], ckv_s[:, m, :])
            nc.tensor.matmul(ss3[:], fr(ones[:]), fr(sq[:]),
                             start=(m == 0), stop=(m == 3))
        r3 = pa.tile([1, TSH], FP)
        nc.scalar.activation(r3[:], ss3[:], AF.Sqrt,
                             bias=epsb[0:1, :], scale=1.0 / KVL)
        nc.vector.reciprocal(r3[:], r3[:])

        # local k rope (rot rows pre-permuted to [A(32) B(32)] on host)
        cck = pa.tile([64, TSH], FP)
        ssk = pa.tile([64, TSH], FP)
        dma(cck[:], cc_k[:])
        dma(ssk[:], ss_k[:])
        kr_sh = pa.tile([64, TSH], FP)
        dma(kr_sh[0:32, :], kr_raw[32:64, :])
        dma(kr_sh[32:64, :], kr_raw[0:32, :])
        nc.vector.tensor_mul(kr_sh[:], kr_sh[:], ssk[:])
        kr = pa.tile([64, TSH], FP)
        nc.vector.tensor_mul(kr[:], kr_raw[:], cck[:])
        nc.vector.tensor_add(kr[:], kr[:], kr_sh[:])

        dma(ag1_in[0:QL, :].rearrange("(m p) t -> p m t", p=128), qa_s[:])
        dma(ag1_in[QL:QL + KVL, :].rearrange("(m p) t -> p m t", p=128),
            ckv_s[:])
        dma(ag1_in[QL + KVL:QL + KVL + DR, :], kr[:])
        dma(ag1_in[2112:2113, :], r2[:])
        dma(ag1_in[2113:2114, :], r3[:])

    nc.gpsimd.collective_compute(
        "AllGather", ALU.bypass, replica_groups=RG,
        ins=[ag1_in.opt()], outs=[ag1_out.opt()])

    # ==================== attention ====================
    with tc.tile_pool(name="att", bufs=1) as at, \
         tc.tile_pool(name="atp", bufs=2) as atp, \
         tc.tile_pool(name="psT", bufs=2, space="PSUM") as pst:

        qn = at.tile([128, 2, T], FR)
        qr = at.tile([128, T], FR)
        qr1 = at.tile([64, T], FR)
        kn = at.tile([128, 2, T], FR)
        krotg = at.tile([64, T], FR)
        vt = at.tile([128, 16, TSH], FR)
        attn = at.tile([128, 2, T], FP)
        maskT = at.tile([128, 4, 512], FP)
        dma(maskT[:], kp(maskT_d))

        with tc.tile_pool(name="proj", bufs=1) as pj, \
             tc.tile_pool(name="projs", bufs=2) as pjs:

            qb_sb = pj.tile([128, 12, HPC * DQK], FR)
            dma(qb_sb[:], kp(qb_wT).bitcast(FR))
            kvb_sb = pj.tile([128, 4, HPC * 256], FR)
            dma(kvb_sb[:], kp(kvb_wT).bitcast(FR))

            b2 = pj.tile([128, T], FP)
            dma(b2[0:1, :], ag1v[:, 2112:2113, :].rearrange("j a t -> a j t"))
            nc.gpsimd.partition_broadcast(b2[:], b2[0:1, :])
            b3 = pj.tile([128, T], FP)
            dma(b3[0:1, :], ag1v[:, 2113:2114, :].rearrange("j a t -> a j t"))
            nc.gpsimd.partition_broadcast(b3[:], b3[0:1, :])
            r3T = pj.tile([128, 16], FP)
            for _n in range(NCORE):
                for _s in range(2):
                    dma(r3T[:, 2 * _n + _s:2 * _n + _s + 1],
                        ag1v[_n, 2113:2114,
                             128 * _s:128 * (_s + 1)].rearrange(
                                 "a t -> t a"))

            dma(krotg[:].rearrange("p (j t) -> p j t", t=TSH),
                ag1v[:, QL + KVL:QL + KVL + DR, :].rearrange(
                    "j p t -> p j t").bitcast(FR))

            for n in range(NCORE):          # 256-token tiles
                nsl = slice(TSH * n, TSH * (n + 1))
                qrhs = pjs.tile([128, 12, TSH], FR, tag="qrhs")
                dma(qrhs[:], ag1v[n, 0:QL, :].rearrange(
                    "(k p) t -> p k t", p=128).bitcast(FR))
                qro = pjs.tile([128, TSH], FP, tag="qro")
                for m in range(3):
                    ps = pst.tile([128, TSH], FP, tag="mm")
                    for k in range(12):
                        nc.tensor.matmul(
                            ps[:], fr(qb_sb[:, k, 128 * m:128 * (m + 1)]),
                            fr(qrhs[:, k, :]),
                            start=(k == 0), stop=(k == 11))
                    dst = qn[:, m, nsl] if m < 2 else qro[:]
                    nc.vector.tensor_mul(dst, ps[:], b2[:, nsl])
                # rope this token tile
                qsh = pjs.tile([128, TSH], FP, tag="qsh")
                dma(qsh[0:32, :], qro[32:64, :])
                dma(qsh[32:64, :], qro[0:32, :])
                dma(qsh[64:96, :], qro[96:128, :])
                dma(qsh[96:128, :], qro[64:96, :])
                ccn = pjs.tile([128, TSH], FP, tag="ccn")
                dma(ccn[:], cc_q[:, nsl])
                ssn = pjs.tile([128, TSH], FP, tag="ssn")
                dma(ssn[:], ss_q[:, nsl])
                nc.vector.tensor_mul(qsh[:], qsh[:], ssn[:])
                nc.vector.tensor_mul(qr[:, nsl], qro[:], ccn[:])
                nc.vector.tensor_add(qr[:, nsl], qr[:, nsl], qsh[:])

                # kv_b for this token tile
                lat_n = pjs.tile([128, 4, TSH], FR, tag="latn")
                dma(lat_n[:], ag1v[n, QL:QL + KVL, :].rearrange(
                    "(k p) t -> p k t", p=128).bitcast(FR))
                for h in range(2):
                    ps = pst.tile([128, TSH], FP, tag="mm")
                    for k in range(4):
                        nc.tensor.matmul(
                            ps[:], fr(kvb_sb[:, k, 128 * h:128 * (h + 1)]),
                            fr(lat_n[:, k, :]),
                            start=(k == 0), stop=(k == 3))
                    nc.vector.tensor_mul(kn[:, h, nsl], ps[:], b3[:, nsl])
                for s2 in range(2):
                    ps = pst.tile([128, TSH], FP, tag="mm")
                    for k in range(4):
                        nc.tensor.matmul(
                            ps[:], fr(lat_n[:, k, 128 * s2:128 * (s2 + 1)]),
                            fr(kvb_sb[:, k, 256:512]),
                            start=(k == 0), stop=(k == 3))
                    sp = 2 * n + s2
                    nc.vector.tensor_scalar(vt[:, sp, :], ps[:],
                                            r3T[:, sp:sp + 1], None,
                                            ALU.mult)

        dma(qr1[:], qr[64:128, :])

        # flash attention, scores transposed [s'_p, s_f]
        for b_ in range(2):
            for h in range(2):
                for sqi in range(2):
                    q0 = 1024 * b_ + 512 * sqi
                    qsl = slice(q0, q0 + 512)
                    nk = 4 * (sqi + 1)
                    aps = pst.tile([128, 512], FP, tag="av")
                    dps = pst.tile([1, 512], FP, tag="dn")
                    for sk in range(nk):
                        k0 = 1024 * b_ + 128 * sk
                        ksl = slice(k0, k0 + 128)
                        sps = pst.tile([128, 512], FP, tag="sc")
                        nc.tensor.matmul(sps[:], fr(kn[:, h, ksl]),
                                         fr(qn[:, h, qsl]),
                                         start=True, stop=False)
                        qrh = qr[0:64, qsl] if h == 0 else qr1[:, qsl]
                        nc.tensor.matmul(
                            sps[:], fr(krotg[:, ksl]), fr(qrh),
                            start=False, stop=True)
                        pr = atp.tile([128, 512], FR, tag="pr", bufs=2)
                        nc.scalar.activation(pr[:], sps[:], AF.Exp,
                                             scale=SCALING)
                        if sk >= 4 * sqi:
                            nc.vector.tensor_mul(
                                pr[:], pr[:], maskT[:, sk - 4 * sqi, :])
                        nc.tensor.matmul(
                            aps[:], fr(vt[:, 8 * b_ + sk,
                                          128 * h:128 * (h + 1)]),
                            fr(pr[:]), start=(sk == 0),
                            stop=(sk == nk - 1), skip_group_check=True)
                        nc.tensor.matmul(
                            dps[:], fr(ones[:]), fr(pr[:]),
                            start=(sk == 0), stop=(sk == nk - 1),
                            skip_group_check=True)
                    rd = atp.tile([1, 512], FP, tag="rd", bufs=1)
                    nc.vector.reciprocal(rd[:], dps[:])
                    rdb = atp.tile([128, 512], FP, tag="rdb", bufs=1)
                    nc.gpsimd.partition_broadcast(rdb[:], rd[:1, :])
                    nc.vector.tensor_mul(attn[:, h, qsl], aps[:], rdb[:])

        a2av = a2a_in.rearrange("(j h p) t -> j p h t", h=2, p=128)
        for j in range(NCORE):
            dma(a2av[j], attn[:, :, TSH * j:TSH * (j + 1)])

    nc.gpsimd.collective_compute(
        "AllToAll", ALU.bypass, replica_groups=RG,
        ins=[a2a_in.opt()], outs=[a2a_out.opt()])

    # ==================== o_proj + ln2 + router ====================
    late_cm = tc.tile_pool(name="late", bufs=1)
    late = late_cm.__enter__()
    x2s = late.tile([128, 16, TSH], FP)
    hs = late.tile([128, 16, TSH], FR)
    bce = late.tile([128, T], FP)

    with tc.tile_pool(name="op", bufs=1) as po, \
         tc.tile_pool(name="opw", bufs=3) as pow_, \
         tc.tile_pool(name="opt", bufs=2) as pot, \
         tc.tile_pool(name="psO", bufs=2, space="PSUM") as pso:

        x0r = po.tile([128, 16, TSH], FP)
        dma(x0r[:], kp(hidT))
        attn_sb = po.tile([128, 16, TSH], FR)
        dma(attn_sb[:], kp(a2a_out[:, :]).bitcast(FR))

        for m in range(16):
            ow = pow_.tile([128, 16, 128], FR, tag="ow")
            dma(ow[:], kp(o_wT[:, 128 * m:128 * (m + 1)]).bitcast(FR))
            ps = pso.tile([128, TSH], FP, tag="mm")
            for k in range(16):
                nc.tensor.matmul(ps[:], fr(ow[:, k, :]),
                                 fr(attn_sb[:, k, :]),
                                 start=(k == 0), stop=(k == 15))
            nc.vector.tensor_add(x2s[:, m, :], ps[:], x0r[:, m, :])

        ss4 = pso.tile([1, TSH], FP, tag="st")
        for k in range(16):
            sq = pot.tile([128, TSH], FR, tag="sq")
            nc.scalar.square(sq[:], x2s[:, k, :])
            nc.tensor.matmul(ss4[:], fr(ones[:]), fr(sq[:]),
                             start=(k == 0), stop=(k == 15))
        r4 = po.tile([1, TSH], FP)
        nc.scalar.activation(r4[:], ss4[:], AF.Sqrt,
                             bias=epsb[0:1, :], scale=1.0 / H)
        nc.vector.reciprocal(r4[:], r4[:])
        b4 = po.tile([128, TSH], FP)
        nc.gpsimd.partition_broadcast(b4[:], r4[:1, :])
        hb = po.tile([128, 16, TSH], BF)
        for m in range(16):
            nc.vector.tensor_mul(hs[:, m, :], x2s[:, m, :], b4[:])
            nc.scalar.copy(hb[:, m, :], hs[:, m, :])
        dma(ag2_in[:, :].rearrange("(m p) t -> p m t", p=128), hb[:])

        # router (fp32 matmuls)
        rw_sb = po.tile([128, 16, NE], FP)
        dma(rw_sb[:], kp(r_wT))
        rb_sb = po.tile([NE, 1], FP)
        dma(rb_sb[:], r_bias[:])
        Gm_s = po.tile([NE, NG], FP)
        dma(Gm_s[:], Gm_d[:])
        Dg_s = po.tile([NG, 16], FP)
        dma(Dg_s[:], Dg_d[:])
        Rg_s = po.tile([16, NG], FP)
        dma(Rg_s[:], Rg_d[:])
        Em_s = po.tile([NG, NE], FP)
        dma(Em_s[:], Em_d[:])
        De_s = po.tile([NE, 64], FP)
        dma(De_s[:], De_d[:])
        Re_s = po.tile([64, NE], FP)
        dma(Re_s[:], Re_d[:])

        lg = pso.tile([NE, TSH], FP, tag="rt")
        for k in range(16):
            nc.tensor.matmul(lg[:], rw_sb[:, k, :], hs[:, k, :].bitcast(FP),
                             start=(k == 0), stop=(k == 15))
        sr = po.tile([NE, TSH], FP)
        nc.scalar.activation(sr[:], lg[:], AF.Sigmoid)
        sc_t = po.tile([NE, TSH], FP)
        nc.vector.tensor_scalar(sc_t[:], sr[:], rb_sb[:, 0:1], None, ALU.add)
        gs_ps = pso.tile([NG, TSH], FP, tag="rt")
        nc.tensor.matmul(gs_ps[:], Gm_s[:], sc_t[:])
        gs_sb = po.tile([NG, TSH], FP)
        nc.scalar.copy(gs_sb[:], gs_ps[:])
        gd_ps = pso.tile([16, TSH], FP, tag="rt")
        nc.tensor.matmul(gd_ps[:], Dg_s[:], gs_sb[:])
        gp = po.tile([16, TSH], FP)
        nc.vector.tensor_scalar(gp[:], gd_ps[:], 0.0, None, ALU.is_gt)
        gc_ps = pso.tile([NG, TSH], FP, tag="rt")
        nc.tensor.matmul(gc_ps[:], Rg_s[:], gp[:])
        gm = po.tile([NG, TSH], FP)
        nc.vector.tensor_scalar(gm[:], gc_ps[:], 2.0, None, ALU.is_lt)
        em_ps = pso.tile([NE, TSH], FP, tag="rt")
        nc.tensor.matmul(em_ps[:], Em_s[:], gm[:])
        msk = po.tile([NE, TSH], FP)
        nc.vector.tensor_mul(msk[:], em_ps[:], sc_t[:])
        ed_ps = pso.tile([64, TSH], FP, tag="rt")
        nc.tensor.matmul(ed_ps[:], De_s[:], msk[:])
        ep = po.tile([64, TSH], FP)
        nc.vector.tensor_scalar(ep[:], ed_ps[:], 0.0, None, ALU.is_gt)
        ec_ps = pso.tile([NE, TSH], FP, tag="rt")
        nc.tensor.matmul(ec_ps[:], Re_s[:], ep[:])
        es = po.tile([NE, TSH], FP)
        nc.vector.tensor_scalar(es[:], ec_ps[:], 2.0, None, ALU.is_lt)
        w_sb = po.tile([NE, TSH], FP)
        nc.vector.tensor_mul(w_sb[:], es[:], sr[:])
        ws_ps = pso.tile([1, TSH], FP, tag="rt")
        nc.tensor.matmul(ws_ps[:], ones[0:NE, :], w_sb[:])
        wse = po.tile([1, TSH], FP)
        nc.vector.tensor_scalar(wse[:], ws_ps[:], 1e-20, None, ALU.add)
        nc.vector.reciprocal(wse[:], wse[:])
        wb = po.tile([NE, TSH], FP)
        nc.gpsimd.partition_broadcast(wb[:], wse[:1, :])
        dw_sb = po.tile([NE, TSH], FP)
        nc.vector.scalar_tensor_tensor(dw_sb[:], w_sb[:], RSF, wb[:],
                                       ALU.mult, ALU.mult)
        dma(agw_in[:, :], dw_sb[:])

        nc.gpsimd.collective_compute(
            "AllGather", ALU.bypass, replica_groups=RG,
            ins=[ag2_in.opt()], outs=[ag2_out.opt()])
        nc.gpsimd.collective_compute(
            "AllGather", ALU.bypass, replica_groups=RG,
            ins=[agw_in.opt()], outs=[agw_out.opt()])

        oh_sb = po.tile([NE, 1], FP)
        dma(oh_sb[:], onehot[:])
        dwg = po.tile([NE, NCORE, TSH], FP)
        dma(dwg[:], agw_out[:, :].rearrange("(j p) t -> p j t", p=NE))
        for jj in range(4):
            ewp = pso.tile([1, 512], FP, tag="rt")
            for q in range(2):
                nc.tensor.matmul(ewp[:, TSH * q:TSH * (q + 1)],
                                 oh_sb[:], dwg[:, 2 * jj + q, :])
            nc.scalar.copy(bce[0:1, 512 * jj:512 * (jj + 1)], ewp[:])
        nc.gpsimd.partition_broadcast(bce[:], bce[0:1, :])

    # ==================== MoE (bf16) + shared expert ====================
    ag2v = ag2_out.rearrange("(j r) t -> j r t", r=H)
    with tc.tile_pool(name="moe", bufs=1) as pm, \
         tc.tile_pool(name="moet", bufs=2) as pmt, \
         tc.tile_pool(name="moew", bufs=2) as pmw, \
         tc.tile_pool(name="psM", bufs=2, space="PSUM") as psm:

        gw_sb = pm.tile([128, 16, INTER], BF)
        dma(gw_sb[:], kp(g_wT))
        uw_sb = pm.tile([128, 16, INTER], BF)
        dma(uw_sb[:], kp(u_wT))
        dwn_sb = pm.tile([128, 6, H], BF)
        dma(dwn_sb[:], kp(d_wT))

        rsv = rs_in.rearrange("(j m p) t -> j m p t", m=16, p=128)
        for n in range(4):
            nsl = slice(512 * n, 512 * (n + 1))
            hb_n = pmt.tile([128, 16, 2, TSH], BF, tag="hb", bufs=1)
            for jj in range(2):
                dma(hb_n[:, :, jj, :],
                    ag2v[2 * n + jj].rearrange("(k p) t -> p k t", p=128))
            act_n = pmt.tile([128, 6, 512], BF, tag="act")
            for m in range(6):
                gp_ = psm.tile([128, 512], FP, tag="mg")
                for k in range(16):
                    nc.tensor.matmul(gp_[:],
                                     gw_sb[:, k, 128 * m:128 * (m + 1)],
                                     hb_n[:, k, :, :],
                                     start=(k == 0), stop=(k == 15))
                gsi = pmt.tile([128, 512], FP, tag="gsi")
                nc.scalar.activation(gsi[:], gp_[:], AF.Sigmoid)
                nc.vector.tensor_mul(gsi[:], gp_[:], gsi[:])
                up_ = psm.tile([128, 512], FP, tag="mg")
                for k in range(16):
                    nc.tensor.matmul(up_[:],
                                     uw_sb[:, k, 128 * m:128 * (m + 1)],
                                     hb_n[:, k, :, :],
                                     start=(k == 0), stop=(k == 15))
                nc.vector.tensor_mul(act_n[:, m, :], up_[:], gsi[:])
            for m in range(16):
                dp = psm.tile([128, 512], FP, tag="md")
                for k in range(6):
                    nc.tensor.matmul(dp[:],
                                     dwn_sb[:, k, 128 * m:128 * (m + 1)],
                                     act_n[:, k, :],
                                     start=(k == 0), stop=(k == 5))
                eo = pmw.tile([128, 512], BF, tag="eo", bufs=3)
                nc.vector.tensor_mul(eo[:], dp[:], bce[:, nsl])
                dma(rsv[2 * n, m], eo[:, 0:TSH])
                dma(rsv[2 * n + 1, m], eo[:, TSH:512])

        # shared expert (f32r, token shard)
        act2 = pm.tile([128, 6, TSH], FR)
        for m in range(6):
            sgw = pmw.tile([128, 16, 128], FR, tag="sgw")
            dma(sgw[:], kp(sg_wT[:, 128 * m:128 * (m + 1)]).bitcast(FR))
            g2 = psm.tile([128, 512], FP, tag="mg")
            for k in range(16):
                nc.tensor.matmul(g2[:, 0:TSH], fr(sgw[:, k, :]),
                                 fr(hs[:, k, :]),
                                 start=(k == 0), stop=(k == 15))
            g2s = pmt.tile([128, TSH], FP, tag="g2s")
            nc.scalar.activation(g2s[:], g2[:, 0:TSH], AF.Sigmoid)
            nc.vector.tensor_mul(g2s[:], g2[:, 0:TSH], g2s[:])
            suw = pmw.tile([128, 16, 128], FR, tag="sgw")
            dma(suw[:], kp(su_wT[:, 128 * m:128 * (m + 1)]).bitcast(FR))
            u2 = psm.tile([128, 512], FP, tag="mg")
            for k in range(16):
                nc.tensor.matmul(u2[:, 0:TSH], fr(suw[:, k, :]),
                                 fr(hs[:, k, :]),
                                 start=(k == 0), stop=(k == 15))
            nc.vector.tensor_mul(act2[:, m, :], u2[:, 0:TSH], g2s[:])
        for m in range(16):
            sdw = pmw.tile([128, 6, 128], FR, tag="sdw")
            dma(sdw[:], kp(sd_wT[:, 128 * m:128 * (m + 1)]).bitcast(FR))
            d2 = psm.tile([128, 512], FP, tag="md")
            for k in range(6):
                nc.tensor.matmul(d2[:, 0:TSH], fr(sdw[:, k, :]),
                                 fr(act2[:, k, :]),
                                 start=(k == 0), stop=(k == 5))
            nc.vector.tensor_add(x2s[:, m, :], d2[:, 0:TSH], x2s[:, m, :])

        nc.gpsimd.collective_compute(
            "ReduceScatter", ALU.add, replica_groups=RG,
            ins=[rs_in.opt()], outs=[rs_out.opt()])

        for m in range(16):
            rsb = pmt.tile([128, TSH], BF, tag="rsb")
            dma(rsb[:], kp(rs_out[:, :])[:, m, :])
            fin = pmt.tile([128, TSH], FP, tag="fin")
            nc.vector.tensor_add(fin[:], rsb[:], x2s[:, m, :])
            dma(out[128 * m:128 * (m + 1), :], fin[:])

    late_cm.__exit__(None, None, None)
    pp_cm.__exit__(None, None, None)
    dram_cm.__exit__(None, None, None)
    tcx.__exit__(None, None, None)

    nc.compile()
    return nc


# --------------------------------------------------------------------------
# host side
# --------------------------------------------------------------------------

_PERM64 = np.concatenate([np.arange(0, 64, 2), np.arange(1, 64, 2)])


def _routing_mats():
    Gm = np.zeros((NE, NG), np.float32)
    for g in range(NG):
        Gm[2 * g, g] = 1.0
        Gm[2 * g + 1, g] = 1.0
    Dg = np.zeros((NG, NG * NG), np.float32)
    Rg = np.zeros((NG * NG, NG), np.float32)
    for i in range(NG):
        for j in range(NG):
            p = i * NG + j
            Dg[i, p] += 1.0
            Dg[j, p] -= 1.0
            Rg[p, j] = 1.0
    Em = np.zeros((NG, NE), np.float32)
    for g in range(NG):
        Em[g, 2 * g] = 1.0
        Em[g, 2 * g + 1] = 1.0
    De = np.zeros((NE, NE * NE), np.float32)
    Re = np.zeros((NE * NE, NE), np.float32)
    for i in range(NE):
        for j in range(NE):
            p = i * NE + j
            De[i, p] += 1.0
            De[j, p] -= 1.0
            Re[p, j] = 1.0
    return Gm, Dg, Rg, Em, De, Re


def _c(a):
    return np.ascontiguousarray(a, dtype=np.float32)


def _bfc(a):
    return np.ascontiguousarray(np.asarray(a, np.float32).astype(
        ml_dtypes.bfloat16))


def make_in_maps(inputs):
    f32 = np.float32
    hs_ = np.asarray(inputs["hidden_states"], f32).reshape(T, H)
    cos = np.asarray(inputs["cos"], f32).reshape(T, DR)
    sin = np.asarray(inputs["sin"], f32).reshape(T, DR)
    ln1 = np.asarray(inputs["ln1_w"], f32)
    ln2 = np.asarray(inputs["ln2_w"], f32)
    qaln = np.asarray(inputs["q_a_ln_w"], f32)
    kvln = np.asarray(inputs["kv_a_ln_w"], f32)

    qa_w = np.asarray(inputs["q_a_w"], f32) * ln1[None, :]
    kva_w = np.asarray(inputs["kv_a_w"], f32) * ln1[None, :]
    kva_w = np.concatenate([kva_w[:KVL], kva_w[KVL:][_PERM64]], 0)
    qb_w = np.asarray(inputs["q_b_w"], f32) * qaln[None, :]
    kvb_w = np.asarray(inputs["kv_b_w"], f32) * kvln[None, :]
    o_w = np.asarray(inputs["o_w"], f32)
    r_w = np.asarray(inputs["router_w"], f32) * ln2[None, :]
    r_b = np.asarray(inputs["router_bias"], f32)
    g_w = np.asarray(inputs["gate_w"], f32) * ln2[None, None, :]
    u_w = np.asarray(inputs["up_w"], f32) * ln2[None, None, :]
    d_w = np.asarray(inputs["down_w"], f32)
    sg_w = np.asarray(inputs["sh_gate_w"], f32) * ln2[None, :]
    su_w = np.asarray(inputs["sh_up_w"], f32) * ln2[None, :]
    sd_w = np.asarray(inputs["sh_down_w"], f32)

    cosT = cos.T
    sinT = sin.T
    cc_q = np.concatenate([cosT[0:32], cosT[32:64]] * 2, 0)
    ss_q = np.concatenate([-sinT[0:32], sinT[32:64]] * 2, 0)
    maskT = np.triu(np.ones((512, 512), np.float32))
    Gm, Dg, Rg, Em, De, Re = _routing_mats()

    shared = dict(
        qa_wT=_c(qa_w.T), kva_wT=_c(kva_w.T), o_wT=_c(o_w.T),
        r_wT=_c(r_w.T), r_bias=_c(r_b.reshape(NE, 1)),
        sg_wT=_c(sg_w.T), su_wT=_c(su_w.T), sd_wT=_c(sd_w.T),
        cc_q=_c(cc_q), ss_q=_c(ss_q), maskT=_c(maskT),
        Gm=_c(Gm), Dg=_c(Dg), Rg=_c(Rg), Em=_c(Em), De=_c(De), Re=_c(Re),
    )

    in_maps = []
    for c in range(NCORE):
        tsl = slice(TSH * c, TSH * (c + 1))
        h0, h1 = 2 * c, 2 * c + 1
        qb_cols = [qb_w[DQK * h0:DQK * h0 + DN],
                   qb_w[DQK * h1:DQK * h1 + DN]]
        for h in (h0, h1):
            rot = qb_w[DQK * h + DN:DQK * (h + 1)]
            qb_cols.append(rot[0::2])
            qb_cols.append(rot[1::2])
        qb_c = np.concatenate(qb_cols, 0)              # [384, QL]
        kvb_c = np.concatenate(
            [kvb_w[256 * h0:256 * h0 + 128],
             kvb_w[256 * h1:256 * h1 + 128],
             kvb_w[256 * h0 + 128:256 * h0 + 256],
             kvb_w[256 * h1 + 128:256 * h1 + 256]], 0)  # [512, KVL]
        oh = np.zeros((NE, 1), np.float32)
        oh[c, 0] = 1.0
        m = dict(shared)
        m.update(
            hidT=_c(hs_[tsl].T),
            qb_wT=_c(qb_c.T), kvb_wT=_c(kvb_c.T),
            cc_k=_c(cosT[:, tsl]),
            ss_k=_c(np.concatenate([-sinT[0:32, tsl],
                                    sinT[32:64, tsl]], 0)),
            onehot=_c(oh),
            g_wT=_bfc(g_w[c].T), u_wT=_bfc(u_w[c].T), d_wT=_bfc(d_w[c].T),
        )
        in_maps.append(m)
    return in_maps


_NC_CACHE = None


def _get_nc():
    global _NC_CACHE
    if _NC_CACHE is None:
        _NC_CACHE = build_program()
    return _NC_CACHE


def kernel(**inputs) -> np.ndarray:
    nc = _get_nc()
    in_maps = make_in_maps(inputs)
    res = bass_utils.run_bass_kernel_spmd(nc, in_maps,
                                          core_ids=list(range(NCORE)))
    full = np.empty((H, T), np.float32)
    for c in range(NCORE):
        full[:, TSH * c:TSH * (c + 1)] = res.results[c]["out"]
    return np.ascontiguousarray(full.T).reshape(B, S, H)



# revision 2
# speedup vs baseline: 1.1861x; 1.1861x over previous
"""DeepseekV3 decoder layer on 8 Trainium2 NeuronCores (Bass/Tile), v2.

Sharding: token shards (256 tok/core) for latents/qkv projections, o_proj,
router, shared expert; tensor-parallel heads (2/core) for attention;
grouped expert-parallel MoE: 2 groups of 4 cores (batch-parallel), 2 routed
experts per core within its group.

Collectives (all bf16, ordered for overlap with compute):
  A2A1a: k_nope/v/k_rot to head-owners (fires early, q path computes under)
  A2A1b: q to head-owners
  A2A2 : attention out back to token-owners
  AG2a : first half of h within 4-core group (fires before router)
  AG2b : second half of h + router weights
  RSx4 : expert partials reduce-scattered in 4 H-chunks, overlapping the
         down-projection; final adds fused per chunk

Weights bf16 in "SBUF-image" layout (one fully-contiguous DMA per stream
chunk); norms, router, residual stream fp32.
"""
import sys

if "/opt/trn_rl_repo" not in sys.path:
    sys.path.insert(0, "/opt/trn_rl_repo")

import numpy as np
import ml_dtypes

import concourse.bass as bass
import concourse.bacc as bacc
import concourse.tile as tile
from concourse import mybir
from concourse import bass_utils

FP = mybir.dt.float32
BF = mybir.dt.bfloat16
FR = mybir.dt.float32r
AF = mybir.ActivationFunctionType
ALU = mybir.AluOpType

NCORE = 8
GSZ = 4
B, S, H = 2, 1024, 2048
T = B * S
GTOK = T // 2
NH, DN, DR, DV = 16, 128, 64, 128
DQK = DN + DR
KVL, QL = 512, 1536
NE, NG, INTER = 8, 4, 768
TSH = T // NCORE           # 256
HPC = NH // NCORE          # 2
SCALING = float(DQK) ** -0.5
RSF = 2.5
EPS = 1e-6

KVROWS = 576               # a2a1a rows per dst: kn 256 + vt 256 + krot 64
QROWS = 384                # a2a1b rows per dst


def fr(ap):
    return ap.bitcast(FR)


def build_program():
    nc = bacc.Bacc("TRN2", target_bir_lowering=False, debug=False,
                   num_devices=NCORE)

    def din(name, shape, dtype=FP):
        return nc.dram_tensor(name, shape, dtype, kind="ExternalInput").ap()

    hidT = din("hidT", [H, TSH])
    qa_wi = din("qa_wi", [128, 192 * 128], BF)       # (m12, k16, 128)
    kva_wi = din("kva_wi", [128, 16 * 576], BF)      # (k16, 576)
    qb_wi = din("qb_wi", [128, 288 * 128], BF)       # (m24, k12, 128)
    kvbk_wi = din("kvbk_wi", [128, 64 * 128], BF)    # (m16, k4, 128)
    kvbv_wi = din("kvbv_wi", [128, 16 * 512], BF)    # (m4, k4, 512)
    o_wi = din("o_wi", [128, 16 * 2048], BF)         # (k16, 2048)
    r_wT = din("r_wT", [H, NE])
    r_bias = din("r_bias", [NE, 1])
    sel = din("sel", [NE, 2])
    g_wi = din("g_wi", [128, 2 * 96 * 128], BF)      # per e: (m6, k16, 128)
    u_wi = din("u_wi", [128, 2 * 96 * 128], BF)
    d_wi = din("d_wi", [128, 2 * 96 * 128], BF)      # per e: (m16, k6, 128)
    sg_wi = din("sg_wi", [128, 96 * 128], BF)        # (m6, k16, 128)
    su_wi = din("su_wi", [128, 96 * 128], BF)
    sd_wi = din("sd_wi", [128, 96 * 128], BF)        # (m16, k6, 128)
    cc_q = din("cc_q", [128, TSH])
    ss_q = din("ss_q", [128, TSH])
    cc_k = din("cc_k", [DR, TSH])
    ss_k = din("ss_k", [DR, TSH])
    maskT_d = din("maskT", [512, 512], BF)
    Gm_d = din("Gm", [NE, NG])
    Dg_d = din("Dg", [NG, NG * NG])
    Rg_d = din("Rg", [NG * NG, NG])
    Em_d = din("Em", [NG, NE])
    De_d = din("De", [NE, NE * NE])
    Re_d = din("Re", [NE * NE, NE])

    out = nc.dram_tensor("out", [H, TSH], FP, kind="ExternalOutput").ap()

    RG8 = [list(range(NCORE))]
    RGG = [[0, 1, 2, 3], [4, 5, 6, 7]]

    def dma(out_ap, in_ap):
        nc.sync.dma_start(out_ap, in_ap)

    def kp(ap, p=128):
        return ap.rearrange("(k p) t -> p k t", p=p)

    tcx = tile.TileContext(nc)
    tc = tcx.__enter__()
    dram_cm = tc.tile_pool(name="dram", bufs=1, space="DRAM")
    dram = dram_cm.__enter__()
    pp_cm = tc.tile_pool(name="persist", bufs=1)
    pp = pp_cm.__enter__()

    a2a1a_in = dram.tile([NCORE * KVROWS, TSH], BF)
    a2a1a_out = dram.tile([NCORE * KVROWS, TSH], BF)
    a2a1b_in = dram.tile([NCORE * QROWS, TSH], BF)
    a2a1b_out = dram.tile([NCORE * QROWS, TSH], BF)
    a2a2_in = dram.tile([NCORE * HPC * DV, TSH], BF)
    a2a2_out = dram.tile([NCORE * HPC * DV, TSH], BF)
    ag2_in = dram.tile([H + NE, TSH], BF)
    ag2_out = dram.tile([GSZ * (H + NE), TSH], BF)
    rs_in = [dram.tile([GSZ * 1024, TSH], BF, name=f"rs_in{r}")
             for r in range(2)]
    rs_out = [dram.tile([1024, TSH], BF, name=f"rs_out{r}")
              for r in range(2)]

    ones = pp.tile([128, 1], FP)
    nc.vector.memset(ones[:], 1.0)
    ones_bf = pp.tile([128, 1], BF)
    nc.vector.memset(ones_bf[:], 1.0)
    ones_row = pp.tile([1, 128], FP)
    nc.vector.memset(ones_row[:], 1.0)
    epsb = pp.tile([128, 1], FP)
    nc.vector.memset(epsb[:], EPS)
    x2s = pp.tile([128, 16, TSH], FP)
    hsb = pp.tile([128, 16, TSH], BF)

    # spans phase A .. o_proj (residual stream input)
    mid_cm = tc.tile_pool(name="mid", bufs=1)
    md = mid_cm.__enter__()
    x0f = md.tile([128, 16, TSH], FP)

    # ==================== phase A: local q/k/v for own tokens ==============
    with tc.tile_pool(name="pA", bufs=1) as pa, \
         tc.tile_pool(name="pAt", bufs=2) as pat, \
         tc.tile_pool(name="psA", bufs=2, space="PSUM") as psa:

        dma(x0f[:], kp(hidT))

        ss_ps = psa.tile([1, TSH], FP, tag="st")
        for k in range(16):
            sq = pat.tile([128, TSH], FR, tag="sq")
            nc.scalar.square(sq[:], x0f[:, k, :])
            nc.tensor.matmul(ss_ps[:], fr(ones[:]), fr(sq[:]),
                             start=(k == 0), stop=(k == 15))
        rstd = pa.tile([1, TSH], FP)
        nc.scalar.activation(rstd[:], ss_ps[:], AF.Sqrt,
                             bias=epsb[0:1, :], scale=1.0 / H)
        nc.vector.reciprocal(rstd[:], rstd[:])
        bloc = psa.tile([128, TSH], FP, tag="bc")
        nc.tensor.matmul(bloc[:], ones_row[0:1, :], rstd[:1, :])
        xb = pa.tile([128, 16, TSH], BF)
        for k in range(16):
            nc.vector.tensor_mul(xb[:, k, :], x0f[:, k, :], bloc[:])

        # ---- kv path first: latent, norm, rope, kv_b, stage + fire A2A1a
        klatf = pa.tile([128, 4, TSH], FP)
        krf = pa.tile([64, TSH], FP)
        with tc.tile_pool(name="wkva", bufs=1) as pw:
            wv = pw.tile([128, 16 * 576], BF)
            dma(wv[:], kva_wi[:])
            for m in range(5):
                mc = 128 if m < 4 else 64
                ps = psa.tile([128, TSH], FP, tag="mm")
                for k in range(16):
                    nc.tensor.matmul(
                        ps[:mc, :], wv[:, k * 576 + 128 * m:
                                       k * 576 + 128 * m + mc],
                        xb[:, k, :], start=(k == 0), stop=(k == 15))
                if m < 4:
                    nc.scalar.copy(klatf[:, m, :], ps[:])
                else:
                    nc.scalar.copy(krf[:], ps[:64, :])

        ss3 = psa.tile([1, TSH], FP, tag="st")
        for m in range(4):
            sq = pat.tile([128, TSH], FR, tag="sq")
            nc.scalar.square(sq[:], klatf[:, m, :])
            nc.tensor.matmul(ss3[:], fr(ones[:]), fr(sq[:]),
                             start=(m == 0), stop=(m == 3))
        r3 = pa.tile([1, TSH], FP)
        nc.scalar.activation(r3[:], ss3[:], AF.Sqrt,
                             bias=epsb[0:1, :], scale=1.0 / KVL)
        nc.vector.reciprocal(r3[:], r3[:])
        b3loc = psa.tile([128, TSH], FP, tag="bc")
        nc.tensor.matmul(b3loc[:], ones_row[0:1, :], r3[:1, :])
        klatb = pa.tile([128, 4, TSH], BF)
        for m in range(4):
            nc.vector.tensor_mul(klatb[:, m, :], klatf[:, m, :], b3loc[:])

        # k rope (rot rows pre-permuted to [evens(32) odds(32)] on host)
        cck = pa.tile([64, TSH], FP)
        ssk = pa.tile([64, TSH], FP)
        dma(cck[:], cc_k[:])
        dma(ssk[:], ss_k[:])
        krsh = pa.tile([64, TSH], FP)
        dma(krsh[0:32, :], krf[32:64, :])
        dma(krsh[32:64, :], krf[0:32, :])
        nc.vector.tensor_mul(krsh[:], krsh[:], ssk[:])
        krc = pa.tile([64, TSH], FP)
        nc.vector.tensor_mul(krc[:], krf[:], cck[:])
        krb = pa.tile([64, TSH], BF)
        nc.vector.tensor_add(krb[:], krc[:], krsh[:])

        kno = pa.tile([128, 16, TSH], BF)
        vto = pa.tile([128, 2, NH * DV], BF)
        with tc.tile_pool(name="wkvb", bufs=1) as pw:
            wk = pw.tile([128, 64 * 128], BF)
            dma(wk[:], kvbk_wi[:])
            for m in range(16):
                ps = psa.tile([128, TSH], FP, tag="mm")
                for k in range(4):
                    nc.tensor.matmul(
                        ps[:], wk[:, (m * 4 + k) * 128:(m * 4 + k + 1) * 128],
                        klatb[:, k, :], start=(k == 0), stop=(k == 3))
                nc.scalar.copy(kno[:, m, :], ps[:])
            wvv = pw.tile([128, 16 * 512], BF)
            dma(wvv[:], kvbv_wi[:])
            for mv in range(4):
                for s2 in range(2):
                    ps = psa.tile([128, 512], FP, tag="mv")
                    for k in range(4):
                        nc.tensor.matmul(
                            ps[:], klatb[:, k, 128 * s2:128 * (s2 + 1)],
                            wvv[:, (mv * 4 + k) * 512:(mv * 4 + k + 1) * 512],
                            start=(k == 0), stop=(k == 3))
                    nc.scalar.copy(vto[:, s2, 512 * mv:512 * (mv + 1)],
                                   ps[:])

        qva = a2a1a_in.rearrange("(j r) t -> j r t", r=KVROWS)
        for j in range(NCORE):
            dma(qva[j, 0:256, :].rearrange("(m p) t -> p m t", p=128),
                kno[:, 2 * j:2 * (j + 1), :])
            dma(qva[j, 256:512, :].rearrange("(s p) t -> p s t", p=128),
                vto[:, :, 256 * j:256 * (j + 1)])
            dma(qva[j, 512:576, :], krb[:])

        nc.gpsimd.collective_compute(
            "AllToAll", ALU.bypass, replica_groups=RG8,
            ins=[a2a1a_in.opt()], outs=[a2a1a_out.opt()])

        # ---- q path: latent, norm, q_b + rope, stage + fire A2A1b
        qlatf = pa.tile([128, 12, TSH], FP)
        with tc.tile_pool(name="wqa", bufs=2) as pw:
            for g in range(2):
                wa = pw.tile([128, 96 * 128], BF, tag="wa")
                dma(wa[:], qa_wi[:, g * 96 * 128:(g + 1) * 96 * 128])
                for mm in range(6):
                    m = 6 * g + mm
                    ps = psa.tile([128, TSH], FP, tag="mm")
                    for k in range(16):
                        nc.tensor.matmul(
                            ps[:], wa[:, (mm * 16 + k) * 128:
                                      (mm * 16 + k + 1) * 128],
                            xb[:, k, :], start=(k == 0), stop=(k == 15))
                    nc.scalar.copy(qlatf[:, m, :], ps[:])

        ss2 = psa.tile([1, TSH], FP, tag="st")
        for m in range(12):
            sq = pat.tile([128, TSH], FR, tag="sq")
            nc.scalar.square(sq[:], qlatf[:, m, :])
            nc.tensor.matmul(ss2[:], fr(ones[:]), fr(sq[:]),
                             start=(m == 0), stop=(m == 11))
        r2 = pa.tile([1, TSH], FP)
        nc.scalar.activation(r2[:], ss2[:], AF.Sqrt,
                             bias=epsb[0:1, :], scale=1.0 / QL)
        nc.vector.reciprocal(r2[:], r2[:])
        b2loc = psa.tile([128, TSH], FP, tag="bc")
        nc.tensor.matmul(b2loc[:], ones_row[0:1, :], r2[:1, :])
        qlatb = pa.tile([128, 12, TSH], BF)
        for m in range(12):
            nc.vector.tensor_mul(qlatb[:, m, :], qlatf[:, m, :], b2loc[:])

        ccq = pa.tile([128, TSH], FP)
        ssq = pa.tile([128, TSH], FP)
        dma(ccq[:], cc_q[:])
        dma(ssq[:], ss_q[:])
        qbo = pa.tile([128, 24, TSH], BF)
        with tc.tile_pool(name="wqb", bufs=2) as pw:
            for g in range(4):
                wb = pw.tile([128, 72 * 128], BF, tag="wb")
                dma(wb[:], qb_wi[:, g * 72 * 128:(g + 1) * 72 * 128])
                for mm in range(6):
                    m = 6 * g + mm
                    ps = psa.tile([128, TSH], FP, tag="mm")
                    for k in range(12):
                        nc.tensor.matmul(
                            ps[:], wb[:, (mm * 12 + k) * 128:
                                      (mm * 12 + k + 1) * 128],
                            qlatb[:, k, :], start=(k == 0), stop=(k == 11))
                    if m % 3 < 2:
                        nc.scalar.copy(qbo[:, m, :], ps[:])
                    else:
                        qro = pat.tile([128, TSH], FP, tag="qro")
                        nc.scalar.copy(qro[:], ps[:])
                        qsh = pat.tile([128, TSH], FP, tag="qsh")
                        dma(qsh[0:32, :], qro[32:64, :])
                        dma(qsh[32:64, :], qro[0:32, :])
                        dma(qsh[64:96, :], qro[96:128, :])
                        dma(qsh[96:128, :], qro[64:96, :])
                        nc.vector.tensor_mul(qsh[:], qsh[:], ssq[:])
                        qrc = pat.tile([128, TSH], FP, tag="qrc")
                        nc.vector.tensor_mul(qrc[:], qro[:], ccq[:])
                        nc.vector.tensor_add(qbo[:, m, :], qrc[:], qsh[:])

        qvb = a2a1b_in.rearrange("(j r) t -> j r t", r=QROWS)
        for j in range(NCORE):
            dma(qvb[j, 0:384, :].rearrange("(m p) t -> p m t", p=128),
                qbo[:, 3 * j:3 * (j + 1), :])

        nc.gpsimd.collective_compute(
            "AllToAll", ALU.bypass, replica_groups=RG8,
            ins=[a2a1b_in.opt()], outs=[a2a1b_out.opt()])

    # ==================== attention (2 heads/core, all tokens) =============
    mid2_cm = tc.tile_pool(name="mid2", bufs=1)
    md2 = mid2_cm.__enter__()
    o_sb = md2.tile([128, 16 * 2048], BF)
    dma(o_sb[:], o_wi[:])

    with tc.tile_pool(name="att", bufs=1) as at, \
         tc.tile_pool(name="atp", bufs=2) as atp, \
         tc.tile_pool(name="psT", bufs=2, space="PSUM") as pst:

        ava = a2a1a_out.rearrange("(n r) t -> n r t", r=KVROWS)
        avb = a2a1b_out.rearrange("(n r) t -> n r t", r=QROWS)
        qn = at.tile([128, 2, T], BF)
        qr = at.tile([128, T], BF)
        kn = at.tile([128, 2, T], BF)
        vt = at.tile([128, 16, TSH], BF)
        krotg = at.tile([64, T], BF)
        for n in range(NCORE):
            tsl = slice(TSH * n, TSH * (n + 1))
            dma(kn[:, :, tsl],
                ava[n, 0:256, :].rearrange("(m p) t -> p m t", p=128))
            dma(vt[:, 2 * n:2 * (n + 1), :],
                ava[n, 256:512, :].rearrange("(s p) t -> p s t", p=128))
            dma(krotg[:, tsl], ava[n, 512:576, :])
            dma(qn[:, :, tsl],
                avb[n, 0:256, :].rearrange("(m p) t -> p m t", p=128))
            dma(qr[:, tsl], avb[n, 256:384, :])
        qr1 = at.tile([64, T], BF)
        dma(qr1[:], qr[64:128, :])
        maskT = at.tile([128, 4, 512], BF)
        dma(maskT[:], kp(maskT_d))
        attn = at.tile([128, 2, T], BF)

        # flash attention, scores transposed [k_p, q_f]
        for b_ in range(2):
            for h in range(2):
                for sqi in range(2):
                    q0 = 1024 * b_ + 512 * sqi
                    qsl = slice(q0, q0 + 512)
                    nk = 4 * (sqi + 1)
                    aps = pst.tile([128, 512], FP, tag="av")
                    dps = pst.tile([1, 512], FP, tag="dn")
                    for sk in range(nk):
                        k0 = 1024 * b_ + 128 * sk
                        ksl = slice(k0, k0 + 128)
                        sps = pst.tile([128, 512], FP, tag="sc",
                                       bufs=3)
                        nc.tensor.matmul(sps[:], kn[:, h, ksl],
                                         qn[:, h, qsl],
                                         start=True, stop=False)
                        qrh = qr[0:64, qsl] if h == 0 else qr1[:, qsl]
                        nc.tensor.matmul(sps[:], krotg[:, ksl], qrh,
                                         start=False, stop=True)
                        pr = atp.tile([128, 512], BF, tag="pr", bufs=3)
                        nc.scalar.activation(pr[:], sps[:], AF.Exp,
                                             scale=SCALING)
                        if sk >= 4 * sqi:
                            nc.vector.tensor_mul(
                                pr[:], pr[:], maskT[:, sk - 4 * sqi, :])
                        nc.tensor.matmul(
                            aps[:], vt[:, 8 * b_ + sk,
                                       128 * h:128 * (h + 1)],
                            pr[:], start=(sk == 0),
                            stop=(sk == nk - 1), skip_group_check=True)
                        nc.tensor.matmul(
                            dps[:], ones_bf[:], pr[:],
                            start=(sk == 0), stop=(sk == nk - 1),
                            skip_group_check=True)
                    rd = atp.tile([1, 512], FP, tag="rd", bufs=1)
                    nc.vector.reciprocal(rd[:], dps[:])
                    rdp = pst.tile([128, 512], FP, tag="bc", bufs=1)
                    nc.tensor.matmul(rdp[:], ones_row[0:1, :], rd[:1, :])
                    rdb = atp.tile([128, 512], FP, tag="rdb", bufs=2)
                    nc.scalar.copy(rdb[:], rdp[:])
                    nc.vector.tensor_mul(attn[:, h, qsl], aps[:], rdb[:])

        a2av = a2a2_in.rearrange("(j h p) t -> j p h t", h=2, p=128)
        for j in range(NCORE):
            dma(a2av[j], attn[:, :, TSH * j:TSH * (j + 1)])

    nc.gpsimd.collective_compute(
        "AllToAll", ALU.bypass, replica_groups=RG8,
        ins=[a2a2_in.opt()], outs=[a2a2_out.opt()])

    # ==================== o_proj + ln2 + router ====================
    with tc.tile_pool(name="op", bufs=1) as po, \
         tc.tile_pool(name="opt", bufs=2) as pot, \
         tc.tile_pool(name="psO", bufs=2, space="PSUM") as pso:

        attn_sb = po.tile([128, 16, TSH], BF)
        dma(attn_sb[:], kp(a2a2_out[:, :]))

        for m in range(16):
            ps = pso.tile([128, TSH], FP, tag="mm")
            for k in range(16):
                nc.tensor.matmul(
                    ps[:], o_sb[:, k * 2048 + 128 * m:k * 2048 + 128 * (m + 1)],
                    attn_sb[:, k, :], start=(k == 0), stop=(k == 15))
            nc.vector.tensor_add(x2s[:, m, :], ps[:], x0f[:, m, :])

        ss4 = pso.tile([1, TSH], FP, tag="st")
        for k in range(16):
            sq = pot.tile([128, TSH], FR, tag="sq")
            nc.scalar.square(sq[:], x2s[:, k, :])
            nc.tensor.matmul(ss4[:], fr(ones[:]), fr(sq[:]),
                             start=(k == 0), stop=(k == 15))
        r4 = po.tile([1, TSH], FP)
        nc.scalar.activation(r4[:], ss4[:], AF.Sqrt,
                             bias=epsb[0:1, :], scale=1.0 / H)
        nc.vector.reciprocal(r4[:], r4[:])
        b4 = pso.tile([128, TSH], FP, tag="bc")
        nc.tensor.matmul(b4[:], ones_row[0:1, :], r4[:1, :])
        hs = po.tile([128, 16, TSH], FP)
        for m in range(16):
            nc.vector.tensor_mul(hs[:, m, :], x2s[:, m, :], b4[:])
            nc.scalar.copy(hsb[:, m, :], hs[:, m, :])
        dma(ag2_in[0:H, :].rearrange("(m p) t -> p m t", p=128), hsb[:])

        # router (fp32)
        rw_sb = po.tile([128, 16, NE], FP)
        dma(rw_sb[:], kp(r_wT))
        rb_sb = po.tile([NE, 1], FP)
        dma(rb_sb[:], r_bias[:])
        Gm_s = po.tile([NE, NG], FP)
        dma(Gm_s[:], Gm_d[:])
        Dg_s = po.tile([NG, 16], FP)
        dma(Dg_s[:], Dg_d[:])
        Rg_s = po.tile([16, NG], FP)
        dma(Rg_s[:], Rg_d[:])
        Em_s = po.tile([NG, NE], FP)
        dma(Em_s[:], Em_d[:])
        De_s = po.tile([NE, 64], FP)
        dma(De_s[:], De_d[:])
        Re_s = po.tile([64, NE], FP)
        dma(Re_s[:], Re_d[:])

        lg = pso.tile([NE, TSH], FP, tag="rt")
        for k in range(16):
            nc.tensor.matmul(lg[:], rw_sb[:, k, :], hs[:, k, :],
                             start=(k == 0), stop=(k == 15))
        sr = po.tile([NE, TSH], FP)
        nc.scalar.activation(sr[:], lg[:], AF.Sigmoid)
        sc_t = po.tile([NE, TSH], FP)
        nc.vector.tensor_scalar(sc_t[:], sr[:], rb_sb[:, 0:1], None, ALU.add)
        gs_ps = pso.tile([NG, TSH], FP, tag="rt")
        nc.tensor.matmul(gs_ps[:], Gm_s[:], sc_t[:])
        gs_sb = po.tile([NG, TSH], FP)
        nc.scalar.copy(gs_sb[:], gs_ps[:])
        gd_ps = pso.tile([16, TSH], FP, tag="rt")
        nc.tensor.matmul(gd_ps[:], Dg_s[:], gs_sb[:])
        gp = po.tile([16, TSH], FP)
        nc.vector.tensor_scalar(gp[:], gd_ps[:], 0.0, None, ALU.is_gt)
        gc_ps = pso.tile([NG, TSH], FP, tag="rt")
        nc.tensor.matmul(gc_ps[:], Rg_s[:], gp[:])
        gm = po.tile([NG, TSH], FP)
        nc.vector.tensor_scalar(gm[:], gc_ps[:], 2.0, None, ALU.is_lt)
        em_ps = pso.tile([NE, TSH], FP, tag="rt")
        nc.tensor.matmul(em_ps[:], Em_s[:], gm[:])
        msk = po.tile([NE, TSH], FP)
        nc.vector.tensor_mul(msk[:], em_ps[:], sc_t[:])
        ed_ps = pso.tile([64, TSH], FP, tag="rt")
        nc.tensor.matmul(ed_ps[:], De_s[:], msk[:])
        ep = po.tile([64, TSH], FP)
        nc.vector.tensor_scalar(ep[:], ed_ps[:], 0.0, None, ALU.is_gt)
        ec_ps = pso.tile([NE, TSH], FP, tag="rt")
        nc.tensor.matmul(ec_ps[:], Re_s[:], ep[:])
        es = po.tile([NE, TSH], FP)
        nc.vector.tensor_scalar(es[:], ec_ps[:], 2.0, None, ALU.is_lt)
        w_sb = po.tile([NE, TSH], FP)
        nc.vector.tensor_mul(w_sb[:], es[:], sr[:])
        ws_ps = pso.tile([1, TSH], FP, tag="rt")
        nc.tensor.matmul(ws_ps[:], ones[0:NE, :], w_sb[:])
        wse = po.tile([1, TSH], FP)
        nc.vector.tensor_scalar(wse[:], ws_ps[:], 1e-20, None, ALU.add)
        nc.vector.reciprocal(wse[:], wse[:])
        wb = pso.tile([NE, TSH], FP, tag="bc")
        nc.tensor.matmul(wb[:], ones_row[0:1, 0:NE], wse[:1, :])
        dw_sb = po.tile([NE, TSH], FP)
        nc.vector.scalar_tensor_tensor(dw_sb[:], w_sb[:], RSF, wb[:],
                                       ALU.mult, ALU.mult)
        dwb = po.tile([NE, TSH], BF)
        nc.scalar.copy(dwb[:], dw_sb[:])
        dma(ag2_in[H:H + NE, :], dwb[:])

        nc.gpsimd.collective_compute(
            "AllGather", ALU.bypass, replica_groups=RGG,
            ins=[ag2_in.opt()], outs=[ag2_out.opt()])

    mid2_cm.__exit__(None, None, None)
    mid_cm.__exit__(None, None, None)

    # ============ MoE: shared expert (overlaps AG2) + 2 routed ===========
    with tc.tile_pool(name="moe", bufs=1) as pm, \
         tc.tile_pool(name="moet", bufs=2) as pmt, \
         tc.tile_pool(name="moew", bufs=2) as pmw, \
         tc.tile_pool(name="psM", bufs=2, space="PSUM") as psm:

        # shared expert on own 256 tokens (no collective dependency)
        act2 = pm.tile([128, 6, TSH], BF)
        for g in range(2):
            sgw = pmw.tile([128, 48 * 128], BF, tag="wg")
            dma(sgw[:], sg_wi[:, g * 48 * 128:(g + 1) * 48 * 128])
            suw = pmw.tile([128, 48 * 128], BF, tag="wu")
            dma(suw[:], su_wi[:, g * 48 * 128:(g + 1) * 48 * 128])
            for mm in range(3):
                m = 3 * g + mm
                g2 = psm.tile([128, TSH], FP, tag="sg")
                for k in range(16):
                    nc.tensor.matmul(
                        g2[:], sgw[:, (mm * 16 + k) * 128:
                                    (mm * 16 + k + 1) * 128],
                        hsb[:, k, :], start=(k == 0), stop=(k == 15))
                g2s = pmt.tile([128, TSH], FP, tag="g2s")
                nc.scalar.activation(g2s[:], g2[:], AF.Sigmoid)
                nc.vector.tensor_mul(g2s[:], g2[:], g2s[:])
                u2 = psm.tile([128, TSH], FP, tag="sg")
                for k in range(16):
                    nc.tensor.matmul(
                        u2[:], suw[:, (mm * 16 + k) * 128:
                                    (mm * 16 + k + 1) * 128],
                        hsb[:, k, :], start=(k == 0), stop=(k == 15))
                nc.vector.tensor_mul(act2[:, m, :], u2[:], g2s[:])
        for g in range(2):
            sdw = pmw.tile([128, 48 * 128], BF, tag=("wg" if g == 0
                                                     else "wu"))
            dma(sdw[:], sd_wi[:, g * 48 * 128:(g + 1) * 48 * 128])
            for mm in range(8):
                m = 8 * g + mm
                d2 = psm.tile([128, TSH], FP, tag="sg")
                for k in range(6):
                    nc.tensor.matmul(
                        d2[:], sdw[:, (mm * 6 + k) * 128:
                                    (mm * 6 + k + 1) * 128],
                        act2[:, k, :], start=(k == 0), stop=(k == 5))
                nc.vector.tensor_add(x2s[:, m, :], d2[:], x2s[:, m, :])

        # gathered h for group tokens: cols (c, t)
        ag2v = ag2_out.rearrange("(c r) t -> c r t", r=H + NE)
        h_sb = pm.tile([128, 16, GTOK], BF)
        dwg_bf = pm.tile([NE, GTOK], BF)
        for c in range(GSZ):
            tsl = slice(TSH * c, TSH * (c + 1))
            dma(h_sb[:, :, tsl],
                ag2v[c, 0:H, :].rearrange("(k p) t -> p k t", p=128))
            dma(dwg_bf[:, tsl], ag2v[c, H:H + NE, :])
        dwgf = pm.tile([NE, GTOK], FP)
        nc.scalar.copy(dwgf[:], dwg_bf[:])
        sel_sb = pm.tile([NE, 2], FP)
        dma(sel_sb[:], sel[:])
        wrow = pm.tile([2, GTOK], FP)
        for q in range(2):
            wps = psm.tile([2, 512], FP, tag="rt", bufs=1)
            nc.tensor.matmul(wps[:], sel_sb[:],
                             dwgf[:, 512 * q:512 * (q + 1)])
            nc.scalar.copy(wrow[:, 512 * q:512 * (q + 1)], wps[:])
        w1row = pm.tile([1, GTOK], FP)
        dma(w1row[:], wrow[1:2, :])
        w0b = pm.tile([128, GTOK], FP)
        w1b = pm.tile([128, GTOK], FP)
        for q in range(2):
            qsl = slice(512 * q, 512 * (q + 1))
            wbp = psm.tile([128, 512], FP, tag="rt", bufs=1)
            nc.tensor.matmul(wbp[:], ones_row[0:1, :], wrow[0:1, qsl])
            nc.scalar.copy(w0b[:, qsl], wbp[:])
            wbp2 = psm.tile([128, 512], FP, tag="rt", bufs=1)
            nc.tensor.matmul(wbp2[:], ones_row[0:1, :], w1row[0:1, qsl])
            nc.scalar.copy(w1b[:, qsl], wbp2[:])

        # gate/up for both experts
        acts = [pm.tile([128, 6, GTOK], BF, name=f"act{e}")
                for e in range(2)]
        for e in range(2):
            for g in range(2):
                gw = pmw.tile([128, 48 * 128], BF, tag="wg")
                dma(gw[:], g_wi[:, (e * 2 + g) * 48 * 128:
                                 (e * 2 + g + 1) * 48 * 128])
                uw = pmw.tile([128, 48 * 128], BF, tag="wu")
                dma(uw[:], u_wi[:, (e * 2 + g) * 48 * 128:
                                 (e * 2 + g + 1) * 48 * 128])
                for mm in range(3):
                    m = 3 * g + mm
                    for a in range(2):
                        asl = slice(512 * a, 512 * (a + 1))
                        gps = psm.tile([128, 512], FP, tag="mg")
                        for k in range(16):
                            nc.tensor.matmul(
                                gps[:], gw[:, (mm * 16 + k) * 128:
                                           (mm * 16 + k + 1) * 128],
                                h_sb[:, k, asl],
                                start=(k == 0), stop=(k == 15))
                        gsi = pmt.tile([128, 512], FP, tag="gsi")
                        nc.scalar.activation(gsi[:], gps[:], AF.Sigmoid)
                        nc.vector.tensor_mul(gsi[:], gps[:], gsi[:])
                        ups = psm.tile([128, 512], FP, tag="mg")
                        for k in range(16):
                            nc.tensor.matmul(
                                ups[:], uw[:, (mm * 16 + k) * 128:
                                           (mm * 16 + k + 1) * 128],
                                h_sb[:, k, asl],
                                start=(k == 0), stop=(k == 15))
                        nc.vector.tensor_mul(acts[e][:, m, asl],
                                             ups[:], gsi[:])

        # down proj, combined over both experts, RS chunked by 4 H-blocks
        wd_t = [None, None]
        for r in range(2):
            eo = pmt.tile([128, 8, GTOK], BF, tag="eo", bufs=1)
            eob = pmt.tile([128, 8, GTOK], BF, tag="eob", bufs=1)
            for mm in range(8):
                m = 8 * r + mm
                if m % 8 == 0:
                    for e in range(2):
                        wd_t[e] = pmw.tile([128, 48 * 128], BF,
                                           name=f"wd{e}",
                                           tag=("wg" if e == 0 else "wu"))
                        dma(wd_t[e][:],
                            d_wi[:, (e * 2 + m // 8) * 48 * 128:
                                 (e * 2 + m // 8 + 1) * 48 * 128])
                mloc = m % 8
                for e in range(2):
                    for a in range(2):
                        asl = slice(512 * a, 512 * (a + 1))
                        dp = psm.tile([128, 512], FP, tag="md")
                        for k in range(6):
                            nc.tensor.matmul(
                                dp[:], wd_t[e][:, (mloc * 6 + k) * 128:
                                               (mloc * 6 + k + 1) * 128],
                                acts[e][:, k, asl],
                                start=(k == 0), stop=(k == 5))
                        if e == 0:
                            nc.vector.tensor_mul(eo[:, mm, asl], dp[:],
                                                 w0b[:, asl])
                        else:
                            tmp = pmt.tile([128, 512], FP, tag="tmp")
                            nc.vector.tensor_mul(tmp[:], dp[:], w1b[:, asl])
                            nc.vector.tensor_add(eob[:, mm, asl],
                                                 eo[:, mm, asl], tmp[:])
            rsv = rs_in[r].rearrange("(c q p) t -> c p q t", q=8, p=128)
            for c in range(GSZ):
                dma(rsv[c], eob[:, :, TSH * c:TSH * (c + 1)])
            nc.gpsimd.collective_compute(
                "ReduceScatter", ALU.add, replica_groups=RGG,
                ins=[rs_in[r].opt()], outs=[rs_out[r].opt()])

            for mm in range(8):
                m = 8 * r + mm
                rsb = pmt.tile([128, TSH], BF, tag="rsb")
                dma(rsb[:], kp(rs_out[r][:, :])[:, mm, :])
                fin = pmt.tile([128, TSH], FP, tag="fin")
                nc.vector.tensor_add(fin[:], rsb[:], x2s[:, m, :])
                dma(out[128 * m:128 * (m + 1), :], fin[:])

    pp_cm.__exit__(None, None, None)
    dram_cm.__exit__(None, None, None)
    tcx.__exit__(None, None, None)

    nc.compile()
    return nc


# --------------------------------------------------------------------------
# host side
# --------------------------------------------------------------------------


def _routing_mats():
    Gm = np.zeros((NE, NG), np.float32)
    for g in range(NG):
        Gm[2 * g, g] = 1.0
        Gm[2 * g + 1, g] = 1.0
    Dg = np.zeros((NG, NG * NG), np.float32)
    Rg = np.zeros((NG * NG, NG), np.float32)
    for i in range(NG):
        for j in range(NG):
            p = i * NG + j
            Dg[i, p] += 1.0
            Dg[j, p] -= 1.0
            Rg[p, j] = 1.0
    Em = np.zeros((NG, NE), np.float32)
    for g in range(NG):
        Em[g, 2 * g] = 1.0
        Em[g, 2 * g + 1] = 1.0
    De = np.zeros((NE, NE * NE), np.float32)
    Re = np.zeros((NE * NE, NE), np.float32)
    for i in range(NE):
        for j in range(NE):
            p = i * NE + j
            De[i, p] += 1.0
            De[j, p] -= 1.0
            Re[p, j] = 1.0
    return Gm, Dg, Rg, Em, De, Re


def _c(a):
    return np.ascontiguousarray(a, dtype=np.float32)


def _bfc(a):
    return np.ascontiguousarray(np.asarray(a, np.float32).astype(
        ml_dtypes.bfloat16))


def _img(wT, w):
    """SBUF-image of wT [K, M]: [128, (M//w)*(K//128)*w], cols (m, k, w)."""
    K, M = wT.shape
    kb = K // 128
    nb = M // w
    a = wT.reshape(kb, 128, M)
    cols = [a[:, :, w * m:w * (m + 1)].transpose(1, 0, 2).reshape(128, -1)
            for m in range(nb)]
    return np.concatenate(cols, axis=1)


def make_in_maps(inputs):
    f32 = np.float32
    hs_ = np.asarray(inputs["hidden_states"], f32).reshape(T, H)
    cos = np.asarray(inputs["cos"], f32).reshape(T, DR)
    sin = np.asarray(inputs["sin"], f32).reshape(T, DR)
    ln1 = np.asarray(inputs["ln1_w"], f32)
    ln2 = np.asarray(inputs["ln2_w"], f32)
    qaln = np.asarray(inputs["q_a_ln_w"], f32)
    kvln = np.asarray(inputs["kv_a_ln_w"], f32)

    qa_w = np.asarray(inputs["q_a_w"], f32) * ln1[None, :]
    kva_w = np.asarray(inputs["kv_a_w"], f32) * ln1[None, :]
    perm64 = np.concatenate([np.arange(0, 64, 2), np.arange(1, 64, 2)])
    kva_w = np.concatenate([kva_w[:KVL], kva_w[KVL:][perm64]], 0)
    qb_w = np.asarray(inputs["q_b_w"], f32) * qaln[None, :]
    kvb_w = np.asarray(inputs["kv_b_w"], f32) * kvln[None, :]
    o_w = np.asarray(inputs["o_w"], f32)
    r_w = np.asarray(inputs["router_w"], f32) * ln2[None, :]
    r_b = np.asarray(inputs["router_bias"], f32)
    g_w = np.asarray(inputs["gate_w"], f32) * ln2[None, None, :]
    u_w = np.asarray(inputs["up_w"], f32) * ln2[None, None, :]
    d_w = np.asarray(inputs["down_w"], f32)
    sg_w = np.asarray(inputs["sh_gate_w"], f32) * ln2[None, :]
    su_w = np.asarray(inputs["sh_up_w"], f32) * ln2[None, :]
    sd_w = np.asarray(inputs["sh_down_w"], f32)

    # q_b columns: per dst core j: h2j nope, h2j+1 nope, rot pair block
    qb_cols = []
    for j in range(NCORE):
        h0, h1 = 2 * j, 2 * j + 1
        qb_cols.append(qb_w[DQK * h0:DQK * h0 + DN])
        qb_cols.append(qb_w[DQK * h1:DQK * h1 + DN])
        for h in (h0, h1):
            rot = qb_w[DQK * h + DN:DQK * (h + 1)]
            qb_cols.append(rot[0::2])
            qb_cols.append(rot[1::2])
    qb_c = np.concatenate(qb_cols, 0)                      # [3072, QL]
    kvbk = np.concatenate([kvb_w[256 * h:256 * h + 128]
                           for h in range(NH)], 0)         # [2048, KVL]
    kvbv = np.concatenate([kvb_w[256 * h + 128:256 * h + 256]
                           for h in range(NH)], 0)         # [2048, KVL]

    cosT = cos.T
    sinT = sin.T
    cc_q = np.concatenate([cosT[0:32], cosT[32:64]] * 2, 0)    # [128, T]
    ss_q = np.concatenate([-sinT[0:32], sinT[32:64]] * 2, 0)
    maskT = np.triu(np.ones((512, 512), np.float32))
    Gm, Dg, Rg, Em, De, Re = _routing_mats()

    shared = dict(
        qa_wi=_bfc(_img(qa_w.T, 128)),
        kva_wi=_bfc(_img(kva_w.T, 576)),
        qb_wi=_bfc(_img(qb_c.T, 128)),
        kvbk_wi=_bfc(_img(kvbk.T, 128)),
        kvbv_wi=_bfc(_img(kvbv.T, 512)),
        o_wi=_bfc(_img(o_w.T, 2048)),
        r_wT=_c(r_w.T), r_bias=_c(r_b.reshape(NE, 1)),
        sg_wi=_bfc(_img(sg_w.T, 128)),
        su_wi=_bfc(_img(su_w.T, 128)),
        sd_wi=_bfc(_img(sd_w.T, 128)),
        maskT=_bfc(maskT),
        Gm=_c(Gm), Dg=_c(Dg), Rg=_c(Rg), Em=_c(Em), De=_c(De), Re=_c(Re),
    )

    in_maps = []
    for c in range(NCORE):
        tsl = slice(TSH * c, TSH * (c + 1))
        e0 = 2 * (c % GSZ)
        selm = np.zeros((NE, 2), np.float32)
        selm[e0, 0] = 1.0
        selm[e0 + 1, 1] = 1.0
        m = dict(shared)
        m.update(
            hidT=_c(hs_[tsl].T),
            cc_q=_c(cc_q[:, tsl]),
            ss_q=_c(ss_q[:, tsl]),
            cc_k=_c(cosT[:, tsl]),
            ss_k=_c(np.concatenate([-sinT[0:32, tsl],
                                    sinT[32:64, tsl]], 0)),
            sel=_c(selm),
            g_wi=_bfc(np.concatenate(
                [_img(g_w[e].T, 128) for e in (e0, e0 + 1)], 1)),
            u_wi=_bfc(np.concatenate(
                [_img(u_w[e].T, 128) for e in (e0, e0 + 1)], 1)),
            d_wi=_bfc(np.concatenate(
                [_img(d_w[e].T, 128) for e in (e0, e0 + 1)], 1)),
        )
        in_maps.append(m)
    return in_maps


_NC_CACHE = None


def _get_nc():
    global _NC_CACHE
    if _NC_CACHE is None:
        _NC_CACHE = build_program()
    return _NC_CACHE


def kernel(**inputs) -> np.ndarray:
    nc = _get_nc()
    in_maps = make_in_maps(inputs)
    res = bass_utils.run_bass_kernel_spmd(nc, in_maps,
                                          core_ids=list(range(NCORE)))
    full = np.empty((H, T), np.float32)
    for c in range(NCORE):
        full[:, TSH * c:TSH * (c + 1)] = res.results[c]["out"]
    return np.ascontiguousarray(full.T).reshape(B, S, H)
